# revision 67
# baseline (speedup 1.0000x reference)
"""Trainium2 Bass kernel for the 7-layer Riptide-style binarized CNN.

Data-parallel over 8 NeuronCores, 64 images/core, groups of g=16 images.

The binarized conv layers (L2-L6) run as fp8 DoubleRow matmuls at the
PE's measured stream peak (~154 TF/s, 0.5 cyc per moving element), so
the kernel is compute-bound at the hardware roofline: 2384 DR matmuls
x ~218 ns = ~520 us of a ~609 us kernel, with the Tensor engine >97%%
busy.  L7 runs non-DR (at its tiny free dim DoubleRow's interleaved
LDWEIGHTS load dominates; plain fp8 is ~6x faster per matmul).

Key structural points:
  - All activation grids use a flat [y, x*16imgs] layout (image index
    innermost).  A conv window over (x, i) is then a single contiguous
    512/256/128-col span, so one matmul covers all 16 images of a row
    chunk and post-ops (pool/sign) batch across all images -> far fewer
    Scalar/Vector instructions.
  - L5/L6 compute exact 8x8 output windows (v1 computed the full 10x10
    padded grid, wasting 56%).
  - DoubleRow everywhere: tap pairs are built with *overlapping* moving
    APs (j-dim stride = inter-tap offset into the same buffer), so no
    shifted second copy is stored and the odd tap out of 9 runs as a
    DR pair with zero weights.  All binary-conv matmuls run at the fp8
    DR rate (0.5 cycles/output element).
  - Maxpool via scalar_tensor_tensor(max) between strided views (cost =
    output size, half of reduce_max), on the Vector engine.
  - L7 + BN + softmax run once over all 64 images at the end.
  - Only pad rings are memset (not whole buffers).
"""

import os
import sys

sys.path.insert(0, "/opt/trn_rl_repo")

import numpy as np
import ml_dtypes
from contextlib import ExitStack

import concourse.bass as bass  # noqa: F401
import concourse.mybir as mybir
import concourse.tile as tile
from concourse import bacc
from concourse.ap import AP
from concourse.bass_utils import run_bass_kernel_spmd
from concourse.masks import make_identity

F32 = mybir.dt.float32
F32R = mybir.dt.float32r
BF16 = mybir.dt.bfloat16
NPBF = ml_dtypes.bfloat16
FP8 = mybir.dt.float8e4
NP8 = ml_dtypes.float8_e4m3fn
DR = mybir.MatmulPerfMode.DoubleRow
MAX = mybir.AluOpType.max
MULT = mybir.AluOpType.mult
AX = mybir.AxisListType.X

NCORES = 8
B = 512
NB = B // NCORES
G = 16
EPS = 1e-3
BIG = 1e30

TAPS9 = [(dy, dx) for dy in range(3) for dx in range(3)]

# DR tap pairs for the 1-kc layers (L2, L3): 4 real pairs + 1 zero-padded.
# ((tap_a), (tap_b or None)) ; moving j-stride = offset(tap_b) - offset(tap_a)
PAIRS = [
    ((0, 0), (0, 1)),
    ((1, 1), (1, 2)),
    ((2, 0), (2, 1)),
    ((0, 2), (1, 0)),
    ((2, 2), None),
]

# row widths (elements) of the flat [y, x*16] grids
W2 = 34 * G   # 544
W3 = 18 * G   # 288
W5 = 10 * G   # 160

# weight block offsets inside the packed fp8 buffer [128, WTOT]
OFF2 = 0
OFF3 = OFF2 + 5 * 2 * 128          # 1280
OFF4 = OFF3 + 2 * 5 * 2 * 128      # 3840
OFF5 = OFF4 + 2 * 9 * 2 * 128      # 8448
OFF6 = OFF5 + 4 * 9 * 2 * 128      # 17664
OFF7 = OFF6 + 4 * 2 * 9 * 2 * 128  # 36096
WTOT = OFF7 + 2 * 16 * 2 * 16      # 38144 (L7 q padded 10->16)

CVCOL = {1: 0, 2: 1, 3: 2, 4: 4, 5: 6, 6: 10}

_prog_cache = {}


def _mov(base_view, off, dims):
    """Raw moving AP: dims = [[stride, size], ...] (free dims, no partition)."""
    pitch = base_view.ap[0][0]
    return AP(base_view.tensor, base_view.offset + off, [[pitch, 128]] + dims)


def build_program(nb=NB, g=G, dump=False):
    assert g == 16 and nb % g == 0
    ngrp = nb // g

    nc = bacc.Bacc("TRN2", target_bir_lowering=False, debug=False)
    Sign = mybir.ActivationFunctionType.Sign
    Exp = mybir.ActivationFunctionType.Exp
    Identity = mybir.ActivationFunctionType.Identity

    xa = nc.declare_dram_parameter("xa", [81, nb * 900], BF16, isOutput=False)
    xb = nc.declare_dram_parameter("xb", [81, nb * 900], BF16, isOutput=False)
    # contiguous copies of the first two output rows: a narrow column
    # slice of the flat layout reads 81 tiny strided rows (~16GB/s), so
    # the first l1 chunk would otherwise wait ~10us for strip (0,8)
    xa0 = nc.declare_dram_parameter("xa0", [81, 960], BF16, isOutput=False)
    xb0 = nc.declare_dram_parameter("xb0", [81, 960], BF16, isOutput=False)
    w1 = nc.declare_dram_parameter("w1", [128, 2, 128], BF16, isOutput=False)
    wall = nc.declare_dram_parameter("wall", [128, WTOT], FP8, isOutput=False)
    cvec = nc.declare_dram_parameter("cvec", [128, 14], F32, isOutput=False)
    bn7 = nc.declare_dram_parameter("bn7", [10, 2], F32, isOutput=False)
    y = nc.declare_dram_parameter("y", [nb, 10], F32, isOutput=True)
    if dump:
        d2 = nc.declare_dram_parameter("d2", [128, 34 * W2 + 16], FP8, isOutput=True)
        d3 = nc.declare_dram_parameter("d3", [128, 18 * W3 + 16], FP8, isOutput=True)
        d4 = nc.declare_dram_parameter("d4", [128, 2 * 18 * W3], FP8, isOutput=True)
        d5 = nc.declare_dram_parameter("d5", [128, 2 * 10 * W5], FP8, isOutput=True)
        d6 = nc.declare_dram_parameter("d6", [128, 4 * 10 * W5], FP8, isOutput=True)
        d7 = nc.declare_dram_parameter("d7", [128, 4 * (nb // g) * 256], FP8, isOutput=True)

    with tile.TileContext(nc) as tc, ExitStack() as ctx:
        consts = ctx.enter_context(tc.tile_pool(name="consts", bufs=1))
        sbufs = ctx.enter_context(tc.tile_pool(name="sbufs", bufs=1))
        xpool = ctx.enter_context(tc.tile_pool(name="xpool", bufs=1))
        post = ctx.enter_context(tc.tile_pool(name="post", bufs=4))
        psum = ctx.enter_context(tc.tile_pool(name="psum", bufs=4, space="PSUM"))

        w1sb = consts.tile([128, 2, 128], BF16)
        nc.sync.dma_start(out=w1sb, in_=w1[:, :, :])
        cv = consts.tile([128, 14], F32)
        nc.sync.dma_start(out=cv, in_=cvec[:, :])
        bn7sb = consts.tile([10, 2], F32)
        nc.sync.dma_start(out=bn7sb, in_=bn7[:, :])
        wsb = consts.tile([128, WTOT], FP8)
        ident = consts.tile([10, 10], F32)
        make_identity(nc, ident)

        # ---- weight views
        w2v = wsb[:, OFF2:OFF3].rearrange("p (k j q) -> p k j q", k=5, j=2, q=128)
        w3v = wsb[:, OFF3:OFF4].rearrange(
            "p (m k j q) -> p m k j q", m=2, k=5, j=2, q=128)
        w4v = wsb[:, OFF4:OFF5].rearrange(
            "p (m t j q) -> p m t j q", m=2, t=9, j=2, q=128)
        w5v = wsb[:, OFF5:OFF6].rearrange(
            "p (m t j q) -> p m t j q", m=4, t=9, j=2, q=128)
        w6v = wsb[:, OFF6:OFF7].rearrange(
            "p (m k t j q) -> p m k t j q", m=4, k=2, t=9, j=2, q=128)
        w7v = wsb[:, OFF7:].rearrange(
            "p (k t j q) -> p k t j q", k=2, t=16, j=2, q=16)

        # ---- activation grids, flat [y, x*16] fp8 (+16 slack for the
        # zero-weight dummy DR windows reading 16 past the end)
        s2 = sbufs.tile([128, 34 * W2 + 16], FP8)
        s3 = sbufs.tile([128, 18 * W3 + 16], FP8)
        s4 = sbufs.tile([128, 2, 18 * W3], FP8)
        s5 = sbufs.tile([128, 2, 10 * W5], FP8)
        s6 = sbufs.tile([128, 4, 10 * W5], FP8)
        s7a = sbufs.tile([128, 4, ngrp, 256], FP8)

        # row views for strided interior writes
        s2r = s2[:, : 34 * W2].rearrange("p (y c) -> p y c", y=34)
        s3r = s3[:, : 18 * W3].rearrange("p (y c) -> p y c", y=18)
        s4r = s4.rearrange("p k (y c) -> p k y c", y=18)
        s5r = s5.rearrange("p k (y c) -> p k y c", y=10)
        s6r = s6.rearrange("p k (y c) -> p k y c", y=10)

        # ---- memset pad rings (+1) and slack once.  s2's pads go on the
        # (prologue-idle) Vector engine so the interleaved early L2 chunks
        # don't wait for the gpsimd queue's identity build.
        nc.vector.memset(s2[:, : 2 * W2], 1.0)
        nc.vector.memset(s2[:, 32 * W2:], 1.0)
        nc.gpsimd.memset(s3[:, : W3], 1.0)
        nc.gpsimd.memset(s3[:, 17 * W3:], 1.0)
        nc.gpsimd.memset(s4[:, :, :W3], 1.0)
        nc.gpsimd.memset(s4[:, :, 17 * W3:], 1.0)
        nc.gpsimd.memset(s5[:, :, :W5], 1.0)
        nc.gpsimd.memset(s5[:, :, 9 * W5:], 1.0)
        nc.gpsimd.memset(s6[:, :, :W5], 1.0)
        nc.gpsimd.memset(s6[:, :, 9 * W5:], 1.0)
        # side columns: rows [pad, rows-pad), x < pad or x >= X-pad
        nc.vector.memset(
            s2r[:, 2:32, 0:2 * G], 1.0)
        nc.vector.memset(
            s2r[:, 2:32, 32 * G:], 1.0)
        nc.gpsimd.memset(
            s3r[:, 1:17, 0:G], 1.0)
        nc.gpsimd.memset(
            s3r[:, 1:17, 17 * G:], 1.0)
        for kk in range(2):
            nc.gpsimd.memset(
                s4r[:, kk, 1:17, 0:G], 1.0)
            nc.gpsimd.memset(
                s4r[:, kk, 1:17, 17 * G:], 1.0)
            nc.gpsimd.memset(
                s5r[:, kk, 1:9, 0:G], 1.0)
            nc.gpsimd.memset(
                s5r[:, kk, 1:9, 9 * G:], 1.0)
        for kk in range(4):
            nc.gpsimd.memset(
                s6r[:, kk, 1:9, 0:G], 1.0)
            nc.gpsimd.memset(
                s6r[:, kk, 1:9, 9 * G:], 1.0)

        def tb(layer, mc=0):
            c = CVCOL[layer] + mc
            return cv[:, c:c + 1]

        # strip row ranges for L1 (PE row tiling, K=27 per strip)
        STRIPS = [(0, 8), (8, 16), (16, 24), (24, 30)]

        # tap-pair (offset, jstride) tables for L2 / L3
        def pair_tab(roww):
            tab = []
            for ta, tEb in PAIRS:
                o = ta[0] * roww + ta[1] * G
                if tEb is None:
                    d = G  # dummy: stride anywhere valid; weights are zero
                else:
                    d = tEb[0] * roww + tEb[1] * G - o
                tab.append((o, d))
            return tab

        P2 = pair_tab(W2)
        P3 = pair_tab(W3)

        xbuf = {}

        def xdma(grp):
            xta = xpool.tile([128, 14400], BF16, tag="xa")
            xtb = xpool.tile([128, 14400], BF16, tag="xb")
            xbuf[grp] = (xta, xtb)
            strips = STRIPS
            if grp == 0:
                # rows 0-1 from the dense side tensors (in-flight on
                # separate DMA rings, land ~2us) so chunk 0 starts early
                nc.sync.dma_start(out=xta[0:81, 0:960], in_=xa0[:, :])
                nc.sync.dma_start(out=xtb[0:81, 0:960], in_=xb0[:, :])
                strips = [(2, 8)] + STRIPS[1:]
            for (y0, y1) in strips:
                nc.sync.dma_start(
                    out=xta[0:81, y0 * 480:y1 * 480],
                    in_=xa[:, grp * 14400 + y0 * 480: grp * 14400 + y1 * 480],
                )
                nc.sync.dma_start(
                    out=xtb[0:81, y0 * 480:y1 * 480],
                    in_=xb[:, grp * 14400 + y0 * 480: grp * 14400 + y1 * 480],
                )

        def l1_chunk(grp, c):
            # conv1 as exact 3-level bf16 split (2 K=81 matmuls per output
            # row: terms x1w1+x1w2+x2w1, then x2w2+x1w3+x3w1)
            xta, xtb = xbuf[grp]
            p = psum.tile([128, 2, 512], F32, tag="p")
            for r in range(2):
                yy = 2 * c + r
                nc.tensor.matmul(
                    p[:, r, 0:480], w1sb[0:81, 0, :],
                    xta[0:81, yy * 480:(yy + 1) * 480],
                    start=True, stop=False,
                )
                nc.tensor.matmul(
                    p[:, r, 0:480], w1sb[0:81, 1, :],
                    xtb[0:81, yy * 480:(yy + 1) * 480],
                    start=False, stop=True,
                )
            nc.scalar.activation(
                s2r[:, 2 + 2 * c:4 + 2 * c, 2 * G:32 * G],
                p[:, :, 0:480], Sign, bias=tb(1), scale=1.0,
            )

        # prologue: x block DMAs first so L1 starts early; the small L2-L4
        # weight piece next (needed ~20us in); the big remainder after.
        xdma(0)
        # the small w2 block lands on its own DMA ring in ~2us so the L2
        # chunks interleaved into the prologue below don't wait for it
        nc.sync.dma_start(out=wsb[:, :OFF3], in_=wall[:, :OFF3])
        nc.sync.dma_start(out=wsb[:, OFF3:OFF5], in_=wall[:, OFF3:OFF5])
        nc.sync.dma_start(out=wsb[:, OFF5:], in_=wall[:, OFF5:])

        # L2: binconv 128->128 (5 DR pairs), pool, sign
        def l2_chunk(grp, c):
            p = psum.tile([128, 2, 512], F32, tag="p", name="p2")
            for r in range(2):
                yy = 2 * c + r
                for k, (o, d) in enumerate(P2):
                    mov = _mov(s2, yy * W2 + o, [[d, 2], [1, 512]])
                    nc.tensor.matmul(
                        p[:, r, :], w2v[:, k, :, :], mov,
                        start=(k == 0), stop=(k == 4), perf_mode=DR,
                    )
            # maxpool 2x2 on raw psum, then sign -> s3 row 1+c interior
            pe = p.rearrange("p y (x two i) -> p y x i two", two=2, i=G)
            t1 = post.tile([128, 2, 256], F32, tag="t1", name="t1c")
            nc.vector.reduce_max(
                t1.rearrange("p y (x i) -> p y x i", i=G), pe, axis=AX)
            t2 = post.tile([128, 256], F32, tag="t2", name="t2c")
            nc.vector.scalar_tensor_tensor(
                t2, t1[:, 0, :], 1.0, t1[:, 1, :], op0=MULT, op1=MAX,
            )
            nc.scalar.activation(
                s3r[:, 1 + c, G:17 * G], t2, Sign, bias=tb(2), scale=1.0,
            )

        # group-0 prologue: interleave L1 with the L2 chunks whose input
        # rows are already signed.  The x transfer (4.7MB, ~24us through
        # the DMA rings) outpaces L1 alone (~16us) but not L1+L2 (~27us),
        # so the PE rides through instead of stalling on strips.
        for c in range(15):
            l1_chunk(0, c)
            if c >= 5:
                l2_chunk(0, c - 5)
        for grp in range(ngrp):
            # chunks 0-9 of group 0's L2 were pre-emitted into the prologue
            for c in range(10 if grp == 0 else 0, 16):
                l2_chunk(grp, c)

            # ---------------- L3: binconv 128->256 (5 DR pairs), sign
            # c-major so L4's first windows (low rows, both kc planes) are
            # signed several chunks before L4 starts.
            for c in range(4):
                for mc in range(2):
                    p = psum.tile([128, 4, 256], F32, tag="p")
                    for h in range(2):
                        yy = 4 * c + 2 * h
                        for k, (o, d) in enumerate(P3):
                            mov = _mov(s3, yy * W3 + o,
                                       [[d, 2], [W3, 2], [1, 256]])
                            nc.tensor.matmul(
                                p[:, 2 * h:2 * h + 2, :], w3v[:, mc, k, :, :],
                                mov, start=(k == 0), stop=(k == 4),
                                perf_mode=DR,
                            )
                    nc.scalar.activation(
                        s4r[:, mc, 1 + 4 * c:5 + 4 * c, G:17 * G],
                        p, Sign, bias=tb(3, mc), scale=1.0,
                    )

            # ---------------- L4: binconv 256->256 (DR over kc), pool, sign
            for c in range(4):
                for mc in range(2):
                    p = psum.tile([128, 4, 256], F32, tag="p")
                    for h in range(2):
                        yy = 4 * c + 2 * h
                        for t, (dy, dx) in enumerate(TAPS9):
                            mov = _mov(s4, (yy + dy) * W3 + dx * G,
                                       [[18 * W3, 2], [W3, 2], [1, 256]])
                            nc.tensor.matmul(
                                p[:, 2 * h:2 * h + 2, :], w4v[:, mc, t, :, :],
                                mov, start=(t == 0), stop=(t == 8),
                                perf_mode=DR,
                            )
                    pe = p.rearrange("p y (x two i) -> p y x i two", two=2, i=G)
                    t1 = post.tile([128, 4, 128], F32, tag="t1")
                    nc.vector.reduce_max(
                        t1.rearrange("p y (x i) -> p y x i", i=G), pe, axis=AX)
                    t1p = t1.rearrange("p (a two) c -> p a two c", two=2)
                    t2 = post.tile([128, 2, 128], F32, tag="t2")
                    nc.vector.scalar_tensor_tensor(
                        t2, t1p[:, :, 0, :], 1.0, t1p[:, :, 1, :],
                        op0=MULT, op1=MAX,
                    )
                    nc.scalar.activation(
                        s5r[:, mc, 1 + 2 * c:3 + 2 * c, G:9 * G],
                        t2, Sign, bias=tb(4, mc), scale=1.0,
                    )

            # ---------------- L5: binconv 256->512 (DR over kc), sign
            for c in range(2):
                for mc in range(4):
                    p = psum.tile([128, 4, 128], F32, tag="p")
                    for t, (dy, dx) in enumerate(TAPS9):
                        mov = _mov(s5, (4 * c + dy) * W5 + dx * G,
                                   [[10 * W5, 2], [W5, 4], [1, 128]])
                        nc.tensor.matmul(
                            p, w5v[:, mc, t, :, :], mov,
                            start=(t == 0), stop=(t == 8), perf_mode=DR,
                        )
                    nc.scalar.activation(
                        s6r[:, mc, 1 + 4 * c:5 + 4 * c, G:9 * G],
                        p, Sign, bias=tb(5, mc), scale=1.0,
                    )

            # ---------------- L6: binconv 512->512 (DR over kc), pool, sign
            # interleave next group's L1 chunks between L6 chunks so the PE
            # keeps streaming through the group boundary while DVE drains.
            if grp + 1 < ngrp:
                xdma(grp + 1)
                pend = [(grp + 1, cc) for cc in range(15)]
            else:
                pend = []
            k6 = 0
            for mc in range(4):
                for c in range(2):
                    p = psum.tile([128, 4, 128], F32, tag="p")
                    k = 0
                    for kp in range(2):
                        for t, (dy, dx) in enumerate(TAPS9):
                            mov = _mov(
                                s6,
                                kp * 2 * 10 * W5 + (4 * c + dy) * W5 + dx * G,
                                [[10 * W5, 2], [W5, 4], [1, 128]])
                            nc.tensor.matmul(
                                p, w6v[:, mc, kp, t, :, :], mov,
                                start=(k == 0), stop=(k == 17), perf_mode=DR,
                            )
                            k += 1
                    pe = p.rearrange("p y (x two i) -> p y x i two", two=2, i=G)
                    t1 = post.tile([128, 4, 64], F32, tag="t1l6")
                    nc.vector.reduce_max(
                        t1.rearrange("p y (x i) -> p y x i", i=G), pe, axis=AX)
                    t1p = t1.rearrange("p (a two) c -> p a two c", two=2)
                    t2 = post.tile([128, 2, 64], F32, tag="t2l6")
                    nc.vector.scalar_tensor_tensor(
                        t2, t1p[:, :, 0, :], 1.0, t1p[:, :, 1, :],
                        op0=MULT, op1=MAX,
                    )
                    nc.scalar.activation(
                        s7a[:, mc, grp, 2 * c * 64:2 * c * 64 + 128],
                        t2, Sign, bias=tb(6, mc), scale=1.0,
                    )
                    k6 += 1
                    if k6 >= 3:
                        for _ in range(3):
                            if pend:
                                l1_chunk(*pend.pop(0))
            while pend:
                l1_chunk(*pend.pop(0))

        if dump:
            for src_t, dst in ((s2, d2), (s3, d3), (s4, d4), (s5, d5),
                               (s6, d6), (s7a, d7)):
                n = src_t.free_size()
                fl = src_t.rearrange(
                    " ".join(["p"] + [chr(97 + i) for i in range(src_t.ndim - 1)])
                    + " -> p (" + " ".join(chr(97 + i) for i in range(src_t.ndim - 1)) + ")"
                ) if src_t.ndim > 2 else src_t
                nc.sync.dma_start(out=dst[:, :], in_=fl)

        # ---------------- L7: binconv 512->10 (4x4) over all 64 images
        p7f = psum.tile([16, nb], F32, tag="p")
        p7 = p7f[0:10, :]
        # non-DR: at this tiny free dim DoubleRow's interleaved LDWEIGHTS
        # dominates (~219 ns/MM); plain fp8 MMs with 16-column weight
        # loads run at ~36 ns/MM.
        k = 0
        for kp in range(2):
            for t in range(16):
                for j in range(2):
                    mov = _mov(s7a, (2 * kp + j) * ngrp * 256 + t * 16,
                               [[256, ngrp], [1, 16]])
                    nc.tensor.matmul(
                        p7f, w7v[:, kp, t, j, :], mov,
                        start=(k == 0), stop=(k == 63),
                    )
                    k += 1
        h7 = post.tile([10, nb], F32, tag="h7")
        nc.vector.tensor_scalar_max(h7, p7, 0.0)
        v7 = post.tile([10, nb], F32, tag="v7")
        nc.scalar.activation(
            v7, h7, Identity, bias=bn7sb[:, 1:2], scale=bn7sb[:, 0:1])
        pt = psum.tile([nb, 10], F32, tag="p")
        nc.tensor.transpose(pt, v7, ident)
        mx = post.tile([nb, 1], F32, tag="mx")
        nc.vector.reduce_max(mx, pt, axis=AX)
        nmx = post.tile([nb, 1], F32, tag="nmx")
        nc.vector.tensor_scalar_mul(nmx, mx, -1.0)
        ex = post.tile([nb, 10], F32, tag="ex")
        nc.scalar.activation(ex, pt, Exp, bias=nmx, scale=1.0)
        sm = post.tile([nb, 1], F32, tag="sm")
        nc.vector.reduce_sum(sm, ex, axis=AX)
        ri = post.tile([nb, 1], F32, tag="ri")
        nc.vector.reciprocal(ri, sm)
        yo = post.tile([nb, 10], F32, tag="yo")
        nc.vector.tensor_scalar_mul(yo, ex, ri)
        nc.sync.dma_start(out=y[:, :], in_=yo)

    nc.compile()
    return nc


# ------------------------------------------------------------------ host prep

def _thresh_bias(gm, be, m, v):
    """bias such that next-layer input = Sign(pre_bn_value + bias)."""
    a = gm.astype(np.float64) / np.sqrt(v.astype(np.float64) + EPS)
    c = be.astype(np.float64) - a * m.astype(np.float64)
    return np.where(c < 0.0, c / a, BIG).astype(np.float32)  # -T = c/a


def _prep_shared(inputs):
    d = {k: np.asarray(v, np.float32) for k, v in inputs.items()}

    sw = {i: np.where(d[f"w{i}"] >= 0, 1.0, -1.0).astype(np.float32)
          for i in range(2, 8)}

    wall = np.zeros((128, WTOT), dtype=NP8)

    # L2: [128, 5 pairs, 2, 128]
    blk = wall[:, OFF2:OFF3].reshape(128, 5, 2, 128)
    for k, (ta, tbp) in enumerate(PAIRS):
        blk[:, k, 0, :] = sw[2][ta[0], ta[1]].astype(NP8)
        if tbp is not None:
            blk[:, k, 1, :] = sw[2][tbp[0], tbp[1]].astype(NP8)
    # L3: [128, 2 mc, 5, 2, 128]
    blk = wall[:, OFF3:OFF4].reshape(128, 2, 5, 2, 128)
    for mc in range(2):
        for k, (ta, tbp) in enumerate(PAIRS):
            blk[:, mc, k, 0, :] = sw[3][ta[0], ta[1], :,
                                        mc * 128:(mc + 1) * 128].astype(NP8)
            if tbp is not None:
                blk[:, mc, k, 1, :] = sw[3][tbp[0], tbp[1], :,
                                            mc * 128:(mc + 1) * 128].astype(NP8)
    # L4: [128, 2 mc, 9, 2 kc, 128]
    blk = wall[:, OFF4:OFF5].reshape(128, 2, 9, 2, 128)
    for mc in range(2):
        for t, (dy, dx) in enumerate(TAPS9):
            for kc in range(2):
                blk[:, mc, t, kc, :] = sw[4][
                    dy, dx, kc * 128:(kc + 1) * 128,
                    mc * 128:(mc + 1) * 128].astype(NP8)
    # L5: [128, 4 mc, 9, 2 kc, 128]
    blk = wall[:, OFF5:OFF6].reshape(128, 4, 9, 2, 128)
    for mc in range(4):
        for t, (dy, dx) in enumerate(TAPS9):
            for kc in range(2):
                blk[:, mc, t, kc, :] = sw[5][
                    dy, dx, kc * 128:(kc + 1) * 128,
                    mc * 128:(mc + 1) * 128].astype(NP8)
    # L6: [128, 4 mc, 2 kp, 9, 2 j, 128]
    blk = wall[:, OFF6:OFF7].reshape(128, 4, 2, 9, 2, 128)
    for mc in range(4):
        for kp in range(2):
            for t, (dy, dx) in enumerate(TAPS9):
                for j in range(2):
                    blk[:, mc, kp, t, j, :] = sw[6][
                        dy, dx, (2 * kp + j) * 128:(2 * kp + j + 1) * 128,
                        mc * 128:(mc + 1) * 128].astype(NP8)
    # L7: [128, 2 kp, 16 tap, 2 j, 16] (cols 10..15 zero)
    blk = wall[:, OFF7:].reshape(128, 2, 16, 2, 16)
    for kp in range(2):
        for ty in range(4):
            for tx in range(4):
                for j in range(2):
                    blk[:, kp, ty * 4 + tx, j, :10] = sw[7][
                        ty, tx,
                        (2 * kp + j) * 128:(2 * kp + j + 1) * 128, :].astype(NP8)

    cvecv = np.zeros((128, 14), dtype=np.float32)
    tb1 = _thresh_bias(d["g1"], d["be1"], d["m1"], d["v1"])
    cvecv[:, 0] = (d["b1"].astype(np.float64) + tb1.astype(np.float64)).astype(
        np.float32)
    MCN = {2: 1, 3: 2, 4: 2, 5: 4, 6: 4}
    for layer in (2, 3, 4, 5, 6):
        t = _thresh_bias(
            d[f"g{layer}"], d[f"be{layer}"], d[f"m{layer}"], d[f"v{layer}"])
        cvecv[:, CVCOL[layer]:CVCOL[layer] + MCN[layer]] = t.reshape(
            MCN[layer], 128).T

    a7 = d["g7"].astype(np.float64) / np.sqrt(d["v7"].astype(np.float64) + EPS)
    c7 = d["be7"].astype(np.float64) - a7 * d["m7"].astype(np.float64)
    bn7v = np.stack([a7.astype(np.float32), c7.astype(np.float32)], axis=1)

    # 3-level bf16 split of w1: exact w = w1+w2+w3
    w1m = d["w1"].reshape(27, 128)
    w1a = w1m.astype(NPBF)
    w1b = (w1m - w1a.astype(np.float32)).astype(NPBF)
    w1c = (w1m - w1a.astype(np.float32) - w1b.astype(np.float32)).astype(NPBF)
    w1r = np.zeros((128, 2, 128), dtype=NPBF)
    # MM1 terms x1w1+x1w2+x2w1 ; MM2 terms x2w2+x1w3+x3w1
    w1r[0:27, 0, :] = w1a
    w1r[27:54, 0, :] = w1b
    w1r[54:81, 0, :] = w1a
    w1r[0:27, 1, :] = w1b
    w1r[27:54, 1, :] = w1c
    w1r[54:81, 1, :] = w1a
    return d, wall, cvecv, bn7v, w1r


def _im2col(x):
    """x [NB,32,32,C?] -> [27, grp*y*x*i] rows (dy,dx,c)."""
    from numpy.lib.stride_tricks import sliding_window_view

    nbl = x.shape[0]
    sw = sliding_window_view(x, (3, 3), axis=(1, 2))  # [NB,30,30,3,3,3](c,wy,wx)
    im = sw.transpose(4, 5, 3, 0, 1, 2)  # [wy,wx,c,NB,y,x]
    im = im.reshape(27, nbl // G, G, 30, 30)
    im = im.transpose(0, 1, 3, 4, 2)     # [27, grp, y, x, i]
    return np.ascontiguousarray(im).reshape(27, nbl * 900)


def _xsplit(x):
    """x [NB,32,32,3] f32 -> xa, xb [81, NB*900] bf16 (3-level split ims)."""
    x1 = x.astype(NPBF)
    x2 = (x - x1.astype(np.float32)).astype(NPBF)
    x3 = (x - x1.astype(np.float32) - x2.astype(np.float32)).astype(NPBF)
    i1 = _im2col(x1.astype(np.float32)).astype(NPBF)
    i2 = _im2col(x2.astype(np.float32)).astype(NPBF)
    i3 = _im2col(x3.astype(np.float32)).astype(NPBF)
    xav = np.concatenate([i1, i1, i2], axis=0)   # MM1 moving [x1;x1;x2]
    xbv = np.concatenate([i2, i1, i3], axis=0)   # MM2 moving [x2;x1;x3]
    return xav, xbv


LAST_RESULTS = None


def kernel(**inputs):
    global LAST_RESULTS
    key = (NB, G)
    if key not in _prog_cache:
        _prog_cache[key] = build_program(NB, G)
    nc = _prog_cache[key]

    d, wall, cvecv, bn7v, w1r = _prep_shared(inputs)

    in_maps = []
    for c in range(NCORES):
        xav, xbv = _xsplit(d["x"][c * NB:(c + 1) * NB])
        in_maps.append(
            {"xa": xav, "xb": xbv,
             "xa0": np.ascontiguousarray(xav[:, 0:960]),
             "xb0": np.ascontiguousarray(xbv[:, 0:960]),
             "w1": w1r, "wall": wall, "cvec": cvecv, "bn7": bn7v})

    trace = bool(int(os.environ.get("KERNEL_TRACE", "0")))
    res = run_bass_kernel_spmd(
        nc, in_maps, core_ids=list(range(NCORES)), trace=trace)
    LAST_RESULTS = res
    out = np.concatenate([res.results[i]["y"] for i in range(NCORES)], axis=0)
    return out.astype(np.float32)



# revision 68
# speedup vs baseline: 1.0252x; 1.0252x over previous
"""Trainium2 Bass kernel for the 7-layer Riptide-style binarized CNN.

Data-parallel over 8 NeuronCores, 64 images/core, groups of g=16 images.

The binarized conv layers (L2-L6) run as fp8 DoubleRow matmuls at the
PE's measured stream peak (~154 TF/s, 0.5 cyc per moving element), so
the kernel is compute-bound at the hardware roofline: 2384 DR matmuls
x ~218 ns = ~520 us of a ~609 us kernel, with the Tensor engine >97%%
busy.  L7 runs non-DR (at its tiny free dim DoubleRow's interleaved
LDWEIGHTS load dominates; plain fp8 is ~6x faster per matmul).

Key structural points:
  - All activation grids use a flat [y, x*16imgs] layout (image index
    innermost).  A conv window over (x, i) is then a single contiguous
    512/256/128-col span, so one matmul covers all 16 images of a row
    chunk and post-ops (pool/sign) batch across all images -> far fewer
    Scalar/Vector instructions.
  - L5/L6 compute exact 8x8 output windows (v1 computed the full 10x10
    padded grid, wasting 56%).
  - DoubleRow everywhere: tap pairs are built with *overlapping* moving
    APs (j-dim stride = inter-tap offset into the same buffer), so no
    shifted second copy is stored and the odd tap out of 9 runs as a
    DR pair with zero weights.  All binary-conv matmuls run at the fp8
    DR rate (0.5 cycles/output element).
  - Maxpool via scalar_tensor_tensor(max) between strided views (cost =
    output size, half of reduce_max), on the Vector engine.
  - L7 + BN + softmax run once over all 64 images at the end.
  - Only pad rings are memset (not whole buffers).
"""

import os
import sys

sys.path.insert(0, "/opt/trn_rl_repo")

import numpy as np
import ml_dtypes
from contextlib import ExitStack

import concourse.bass as bass  # noqa: F401
import concourse.mybir as mybir
import concourse.tile as tile
from concourse import bacc
from concourse.ap import AP
from concourse.bass_utils import run_bass_kernel_spmd
from concourse.masks import make_identity

F32 = mybir.dt.float32
F32R = mybir.dt.float32r
BF16 = mybir.dt.bfloat16
NPBF = ml_dtypes.bfloat16
FP8 = mybir.dt.float8e4
NP8 = ml_dtypes.float8_e4m3fn
DR = mybir.MatmulPerfMode.DoubleRow
MAX = mybir.AluOpType.max
MULT = mybir.AluOpType.mult
AX = mybir.AxisListType.X

NCORES = 8
B = 512
NB = B // NCORES
G = 16
EPS = 1e-3
BIG = 1e30

TAPS9 = [(dy, dx) for dy in range(3) for dx in range(3)]

# DR tap pairs for the 1-kc layers (L2, L3): 4 real pairs + 1 zero-padded.
# ((tap_a), (tap_b or None)) ; moving j-stride = offset(tap_b) - offset(tap_a)
PAIRS = [
    ((0, 0), (0, 1)),
    ((1, 1), (1, 2)),
    ((2, 0), (2, 1)),
    ((0, 2), (1, 0)),
    ((2, 2), None),
]

# row widths (elements) of the flat [y, x*16] grids
W2 = 34 * G   # 544
W3 = 18 * G   # 288
W5 = 10 * G   # 160

# weight block offsets inside the packed fp8 buffer [128, WTOT]
OFF2 = 0
OFF3 = OFF2 + 5 * 2 * 128          # 1280
OFF4 = OFF3 + 2 * 5 * 2 * 128      # 3840
OFF5 = OFF4 + 2 * 9 * 2 * 128      # 8448
OFF6 = OFF5 + 4 * 9 * 2 * 128      # 17664
OFF7 = OFF6 + 4 * 2 * 9 * 2 * 128  # 36096
WTOT = OFF7 + 2 * 16 * 2 * 16      # 38144 (L7 q padded 10->16)

CVCOL = {1: 0, 2: 1, 3: 2, 4: 4, 5: 6, 6: 10}

_prog_cache = {}


def _mov(base_view, off, dims):
    """Raw moving AP: dims = [[stride, size], ...] (free dims, no partition)."""
    pitch = base_view.ap[0][0]
    return AP(base_view.tensor, base_view.offset + off, [[pitch, 128]] + dims)


def build_program(nb=NB, g=G, dump=False):
    assert g == 16 and nb % g == 0
    ngrp = nb // g

    nc = bacc.Bacc("TRN2", target_bir_lowering=False, debug=False)
    Sign = mybir.ActivationFunctionType.Sign
    Exp = mybir.ActivationFunctionType.Exp
    Identity = mybir.ActivationFunctionType.Identity

    xa = nc.declare_dram_parameter("xa", [81, nb * 900], BF16, isOutput=False)
    xb = nc.declare_dram_parameter("xb", [81, nb * 900], BF16, isOutput=False)
    # contiguous copies of the first two output rows: a narrow column
    # slice of the flat layout reads 81 tiny strided rows (~16GB/s), so
    # the first l1 chunk would otherwise wait ~10us for strip (0,8)
    xa0 = nc.declare_dram_parameter("xa0", [81, 960], BF16, isOutput=False)
    xb0 = nc.declare_dram_parameter("xb0", [81, 960], BF16, isOutput=False)
    w1 = nc.declare_dram_parameter("w1", [128, 2, 128], BF16, isOutput=False)
    wall = nc.declare_dram_parameter("wall", [128, WTOT], FP8, isOutput=False)
    cvec = nc.declare_dram_parameter("cvec", [128, 14], F32, isOutput=False)
    bn7 = nc.declare_dram_parameter("bn7", [10, 2], F32, isOutput=False)
    y = nc.declare_dram_parameter("y", [nb, 10], F32, isOutput=True)
    if dump:
        d2 = nc.declare_dram_parameter("d2", [128, 34 * W2 + 16], FP8, isOutput=True)
        d3 = nc.declare_dram_parameter("d3", [128, 18 * W3 + 16], FP8, isOutput=True)
        d4 = nc.declare_dram_parameter("d4", [128, 2 * 18 * W3], FP8, isOutput=True)
        d5 = nc.declare_dram_parameter("d5", [128, 2 * 10 * W5], FP8, isOutput=True)
        d6 = nc.declare_dram_parameter("d6", [128, 4 * 10 * W5], FP8, isOutput=True)
        d7 = nc.declare_dram_parameter("d7", [128, 4 * (nb // g) * 256], FP8, isOutput=True)

    with tile.TileContext(nc) as tc, ExitStack() as ctx:
        consts = ctx.enter_context(tc.tile_pool(name="consts", bufs=1))
        sbufs = ctx.enter_context(tc.tile_pool(name="sbufs", bufs=1))
        xpool = ctx.enter_context(tc.tile_pool(name="xpool", bufs=1))
        post = ctx.enter_context(tc.tile_pool(name="post", bufs=4))
        psum = ctx.enter_context(tc.tile_pool(name="psum", bufs=4, space="PSUM"))

        w1sb = consts.tile([128, 2, 128], BF16)
        nc.sync.dma_start(out=w1sb, in_=w1[:, :, :])
        cv = consts.tile([128, 14], F32)
        nc.sync.dma_start(out=cv, in_=cvec[:, :])
        bn7sb = consts.tile([10, 2], F32)
        nc.sync.dma_start(out=bn7sb, in_=bn7[:, :])
        wsb = consts.tile([128, WTOT], FP8)
        ident = consts.tile([10, 10], F32)
        make_identity(nc, ident)

        # ---- weight views
        w2v = wsb[:, OFF2:OFF3].rearrange("p (k j q) -> p k j q", k=5, j=2, q=128)
        w3v = wsb[:, OFF3:OFF4].rearrange(
            "p (m k j q) -> p m k j q", m=2, k=5, j=2, q=128)
        w4v = wsb[:, OFF4:OFF5].rearrange(
            "p (m t j q) -> p m t j q", m=2, t=9, j=2, q=128)
        w5v = wsb[:, OFF5:OFF6].rearrange(
            "p (m t j q) -> p m t j q", m=4, t=9, j=2, q=128)
        w6v = wsb[:, OFF6:OFF7].rearrange(
            "p (m k t j q) -> p m k t j q", m=4, k=2, t=9, j=2, q=128)
        w7v = wsb[:, OFF7:].rearrange(
            "p (k t j q) -> p k t j q", k=2, t=16, j=2, q=16)

        # ---- activation grids, flat [y, x*16] fp8 (+16 slack for the
        # zero-weight dummy DR windows reading 16 past the end)
        s2 = sbufs.tile([128, 34 * W2 + 16], FP8)
        s3 = sbufs.tile([128, 18 * W3 + 16], FP8)
        s4 = sbufs.tile([128, 2, 18 * W3], FP8)
        s5 = sbufs.tile([128, 2, 10 * W5], FP8)
        s6 = sbufs.tile([128, 4, 10 * W5], FP8)
        s7a = sbufs.tile([128, 4, ngrp, 256], FP8)

        # row views for strided interior writes
        s2r = s2[:, : 34 * W2].rearrange("p (y c) -> p y c", y=34)
        s3r = s3[:, : 18 * W3].rearrange("p (y c) -> p y c", y=18)
        s4r = s4.rearrange("p k (y c) -> p k y c", y=18)
        s5r = s5.rearrange("p k (y c) -> p k y c", y=10)
        s6r = s6.rearrange("p k (y c) -> p k y c", y=10)

        # ---- memset pad rings (+1) and slack once
        for t, rows, w, pad in (
            (s2, 34, W2, 2), (s3, 18, W3, 1),
        ):
            nc.gpsimd.memset(t[:, : pad * w], 1.0)                    # top
            nc.gpsimd.memset(t[:, (rows - pad) * w:], 1.0)            # bottom+slack
        nc.gpsimd.memset(s4[:, :, :W3], 1.0)
        nc.gpsimd.memset(s4[:, :, 17 * W3:], 1.0)
        nc.gpsimd.memset(s5[:, :, :W5], 1.0)
        nc.gpsimd.memset(s5[:, :, 9 * W5:], 1.0)
        nc.gpsimd.memset(s6[:, :, :W5], 1.0)
        nc.gpsimd.memset(s6[:, :, 9 * W5:], 1.0)
        # side columns: rows [pad, rows-pad), x < pad or x >= X-pad
        nc.gpsimd.memset(
            s2r[:, 2:32, 0:2 * G], 1.0)
        nc.gpsimd.memset(
            s2r[:, 2:32, 32 * G:], 1.0)
        nc.gpsimd.memset(
            s3r[:, 1:17, 0:G], 1.0)
        nc.gpsimd.memset(
            s3r[:, 1:17, 17 * G:], 1.0)
        for kk in range(2):
            nc.gpsimd.memset(
                s4r[:, kk, 1:17, 0:G], 1.0)
            nc.gpsimd.memset(
                s4r[:, kk, 1:17, 17 * G:], 1.0)
            nc.gpsimd.memset(
                s5r[:, kk, 1:9, 0:G], 1.0)
            nc.gpsimd.memset(
                s5r[:, kk, 1:9, 9 * G:], 1.0)
        for kk in range(4):
            nc.gpsimd.memset(
                s6r[:, kk, 1:9, 0:G], 1.0)
            nc.gpsimd.memset(
                s6r[:, kk, 1:9, 9 * G:], 1.0)

        def tb(layer, mc=0):
            c = CVCOL[layer] + mc
            return cv[:, c:c + 1]

        # strip row ranges for L1 (PE row tiling, K=27 per strip)
        STRIPS = [(0, 8), (8, 16), (16, 24), (24, 30)]

        # tap-pair (offset, jstride) tables for L2 / L3
        def pair_tab(roww):
            tab = []
            for ta, tEb in PAIRS:
                o = ta[0] * roww + ta[1] * G
                if tEb is None:
                    d = G  # dummy: stride anywhere valid; weights are zero
                else:
                    d = tEb[0] * roww + tEb[1] * G - o
                tab.append((o, d))
            return tab

        P2 = pair_tab(W2)
        P3 = pair_tab(W3)

        xbuf = {}

        def xdma(grp):
            xta = xpool.tile([128, 14400], BF16, tag="xa")
            xtb = xpool.tile([128, 14400], BF16, tag="xb")
            xbuf[grp] = (xta, xtb)
            strips = STRIPS
            if grp == 0:
                # rows 0-1 from the dense side tensors (in-flight on
                # separate DMA rings, land ~2us) so chunk 0 starts early
                nc.sync.dma_start(out=xta[0:81, 0:960], in_=xa0[:, :])
                nc.sync.dma_start(out=xtb[0:81, 0:960], in_=xb0[:, :])
                strips = [(2, 8)] + STRIPS[1:]
            for (y0, y1) in strips:
                nc.sync.dma_start(
                    out=xta[0:81, y0 * 480:y1 * 480],
                    in_=xa[:, grp * 14400 + y0 * 480: grp * 14400 + y1 * 480],
                )
                nc.sync.dma_start(
                    out=xtb[0:81, y0 * 480:y1 * 480],
                    in_=xb[:, grp * 14400 + y0 * 480: grp * 14400 + y1 * 480],
                )

        def l1_chunk(grp, c):
            # conv1 as exact 3-level bf16 split (2 K=81 matmuls per output
            # row: terms x1w1+x1w2+x2w1, then x2w2+x1w3+x3w1)
            xta, xtb = xbuf[grp]
            p = psum.tile([128, 2, 512], F32, tag="p")
            for r in range(2):
                yy = 2 * c + r
                nc.tensor.matmul(
                    p[:, r, 0:480], w1sb[0:81, 0, :],
                    xta[0:81, yy * 480:(yy + 1) * 480],
                    start=True, stop=False,
                )
                nc.tensor.matmul(
                    p[:, r, 0:480], w1sb[0:81, 1, :],
                    xtb[0:81, yy * 480:(yy + 1) * 480],
                    start=False, stop=True,
                )
            nc.scalar.activation(
                s2r[:, 2 + 2 * c:4 + 2 * c, 2 * G:32 * G],
                p[:, :, 0:480], Sign, bias=tb(1), scale=1.0,
            )

        # prologue: x block DMAs first so L1 starts early; the small L2-L4
        # weight piece next (needed ~20us in); the big remainder after.
        xdma(0)
        nc.sync.dma_start(out=wsb[:, :OFF5], in_=wall[:, :OFF5])
        nc.sync.dma_start(out=wsb[:, OFF5:], in_=wall[:, OFF5:])
        for c in range(15):
            l1_chunk(0, c)
        for grp in range(ngrp):
            # ---------------- L2: binconv 128->128 (5 DR pairs), pool, sign
            for c in range(16):
                p = psum.tile([128, 2, 512], F32, tag="p")
                for r in range(2):
                    yy = 2 * c + r
                    for k, (o, d) in enumerate(P2):
                        mov = _mov(s2, yy * W2 + o, [[d, 2], [1, 512]])
                        nc.tensor.matmul(
                            p[:, r, :], w2v[:, k, :, :], mov,
                            start=(k == 0), stop=(k == 4), perf_mode=DR,
                        )
                # maxpool 2x2 on raw psum, then sign -> s3 row 1+c interior
                pe = p.rearrange("p y (x two i) -> p y x i two", two=2, i=G)
                t1 = post.tile([128, 2, 256], F32, tag="t1")
                nc.vector.reduce_max(
                    t1.rearrange("p y (x i) -> p y x i", i=G), pe, axis=AX)
                t2 = post.tile([128, 256], F32, tag="t2")
                nc.vector.scalar_tensor_tensor(
                    t2, t1[:, 0, :], 1.0, t1[:, 1, :], op0=MULT, op1=MAX,
                )
                nc.scalar.activation(
                    s3r[:, 1 + c, G:17 * G], t2, Sign, bias=tb(2), scale=1.0,
                )

            # ---------------- L3: binconv 128->256 (5 DR pairs), sign
            # c-major so L4's first windows (low rows, both kc planes) are
            # signed several chunks before L4 starts.
            for c in range(4):
                for mc in range(2):
                    p = psum.tile([128, 4, 256], F32, tag="p")
                    for h in range(2):
                        yy = 4 * c + 2 * h
                        for k, (o, d) in enumerate(P3):
                            mov = _mov(s3, yy * W3 + o,
                                       [[d, 2], [W3, 2], [1, 256]])
                            nc.tensor.matmul(
                                p[:, 2 * h:2 * h + 2, :], w3v[:, mc, k, :, :],
                                mov, start=(k == 0), stop=(k == 4),
                                perf_mode=DR,
                            )
                    nc.scalar.activation(
                        s4r[:, mc, 1 + 4 * c:5 + 4 * c, G:17 * G],
                        p, Sign, bias=tb(3, mc), scale=1.0,
                    )

            # ---------------- L4: binconv 256->256 (DR over kc), pool, sign
            for c in range(4):
                for mc in range(2):
                    p = psum.tile([128, 4, 256], F32, tag="p")
                    for h in range(2):
                        yy = 4 * c + 2 * h
                        for t, (dy, dx) in enumerate(TAPS9):
                            mov = _mov(s4, (yy + dy) * W3 + dx * G,
                                       [[18 * W3, 2], [W3, 2], [1, 256]])
                            nc.tensor.matmul(
                                p[:, 2 * h:2 * h + 2, :], w4v[:, mc, t, :, :],
                                mov, start=(t == 0), stop=(t == 8),
                                perf_mode=DR,
                            )
                    pe = p.rearrange("p y (x two i) -> p y x i two", two=2, i=G)
                    t1 = post.tile([128, 4, 128], F32, tag="t1")
                    nc.vector.reduce_max(
                        t1.rearrange("p y (x i) -> p y x i", i=G), pe, axis=AX)
                    t1p = t1.rearrange("p (a two) c -> p a two c", two=2)
                    t2 = post.tile([128, 2, 128], F32, tag="t2")
                    nc.vector.scalar_tensor_tensor(
                        t2, t1p[:, :, 0, :], 1.0, t1p[:, :, 1, :],
                        op0=MULT, op1=MAX,
                    )
                    nc.scalar.activation(
                        s5r[:, mc, 1 + 2 * c:3 + 2 * c, G:9 * G],
                        t2, Sign, bias=tb(4, mc), scale=1.0,
                    )

            # ---------------- L5: binconv 256->512 (DR over kc), sign
            for c in range(2):
                for mc in range(4):
                    p = psum.tile([128, 4, 128], F32, tag="p")
                    for t, (dy, dx) in enumerate(TAPS9):
                        mov = _mov(s5, (4 * c + dy) * W5 + dx * G,
                                   [[10 * W5, 2], [W5, 4], [1, 128]])
                        nc.tensor.matmul(
                            p, w5v[:, mc, t, :, :], mov,
                            start=(t == 0), stop=(t == 8), perf_mode=DR,
                        )
                    nc.scalar.activation(
                        s6r[:, mc, 1 + 4 * c:5 + 4 * c, G:9 * G],
                        p, Sign, bias=tb(5, mc), scale=1.0,
                    )

            # ---------------- L6: binconv 512->512 (DR over kc), pool, sign
            # interleave next group's L1 chunks between L6 chunks so the PE
            # keeps streaming through the group boundary while DVE drains.
            if grp + 1 < ngrp:
                xdma(grp + 1)
                pend = [(grp + 1, cc) for cc in range(15)]
            else:
                pend = []
            k6 = 0
            for mc in range(4):
                for c in range(2):
                    p = psum.tile([128, 4, 128], F32, tag="p")
                    k = 0
                    for kp in range(2):
                        for t, (dy, dx) in enumerate(TAPS9):
                            mov = _mov(
                                s6,
                                kp * 2 * 10 * W5 + (4 * c + dy) * W5 + dx * G,
                                [[10 * W5, 2], [W5, 4], [1, 128]])
                            nc.tensor.matmul(
                                p, w6v[:, mc, kp, t, :, :], mov,
                                start=(k == 0), stop=(k == 17), perf_mode=DR,
                            )
                            k += 1
                    pe = p.rearrange("p y (x two i) -> p y x i two", two=2, i=G)
                    t1 = post.tile([128, 4, 64], F32, tag="t1l6")
                    nc.vector.reduce_max(
                        t1.rearrange("p y (x i) -> p y x i", i=G), pe, axis=AX)
                    t1p = t1.rearrange("p (a two) c -> p a two c", two=2)
                    t2 = post.tile([128, 2, 64], F32, tag="t2l6")
                    nc.vector.scalar_tensor_tensor(
                        t2, t1p[:, :, 0, :], 1.0, t1p[:, :, 1, :],
                        op0=MULT, op1=MAX,
                    )
                    nc.scalar.activation(
                        s7a[:, mc, grp, 2 * c * 64:2 * c * 64 + 128],
                        t2, Sign, bias=tb(6, mc), scale=1.0,
                    )
                    k6 += 1
                    if k6 >= 3:
                        for _ in range(3):
                            if pend:
                                l1_chunk(*pend.pop(0))
            while pend:
                l1_chunk(*pend.pop(0))

        if dump:
            for src_t, dst in ((s2, d2), (s3, d3), (s4, d4), (s5, d5),
                               (s6, d6), (s7a, d7)):
                n = src_t.free_size()
                fl = src_t.rearrange(
                    " ".join(["p"] + [chr(97 + i) for i in range(src_t.ndim - 1)])
                    + " -> p (" + " ".join(chr(97 + i) for i in range(src_t.ndim - 1)) + ")"
                ) if src_t.ndim > 2 else src_t
                nc.sync.dma_start(out=dst[:, :], in_=fl)

        # ---------------- L7: binconv 512->10 (4x4) over all 64 images
        p7f = psum.tile([16, nb], F32, tag="p")
        p7 = p7f[0:10, :]
        # non-DR: at this tiny free dim DoubleRow's interleaved LDWEIGHTS
        # dominates (~219 ns/MM); plain fp8 MMs with 16-column weight
        # loads run at ~36 ns/MM.
        k = 0
        for kp in range(2):
            for t in range(16):
                for j in range(2):
                    mov = _mov(s7a, (2 * kp + j) * ngrp * 256 + t * 16,
                               [[256, ngrp], [1, 16]])
                    nc.tensor.matmul(
                        p7f, w7v[:, kp, t, j, :], mov,
                        start=(k == 0), stop=(k == 63),
                    )
                    k += 1
        h7 = post.tile([10, nb], F32, tag="h7")
        nc.vector.tensor_scalar_max(h7, p7, 0.0)
        v7 = post.tile([10, nb], F32, tag="v7")
        nc.scalar.activation(
            v7, h7, Identity, bias=bn7sb[:, 1:2], scale=bn7sb[:, 0:1])
        pt = psum.tile([nb, 10], F32, tag="p")
        nc.tensor.transpose(pt, v7, ident)
        mx = post.tile([nb, 1], F32, tag="mx")
        nc.vector.reduce_max(mx, pt, axis=AX)
        nmx = post.tile([nb, 1], F32, tag="nmx")
        nc.vector.tensor_scalar_mul(nmx, mx, -1.0)
        ex = post.tile([nb, 10], F32, tag="ex")
        nc.scalar.activation(ex, pt, Exp, bias=nmx, scale=1.0)
        sm = post.tile([nb, 1], F32, tag="sm")
        nc.vector.reduce_sum(sm, ex, axis=AX)
        ri = post.tile([nb, 1], F32, tag="ri")
        nc.vector.reciprocal(ri, sm)
        yo = post.tile([nb, 10], F32, tag="yo")
        nc.vector.tensor_scalar_mul(yo, ex, ri)
        nc.sync.dma_start(out=y[:, :], in_=yo)

    nc.compile()
    return nc


# ------------------------------------------------------------------ host prep

def _thresh_bias(gm, be, m, v):
    """bias such that next-layer input = Sign(pre_bn_value + bias)."""
    a = gm.astype(np.float64) / np.sqrt(v.astype(np.float64) + EPS)
    c = be.astype(np.float64) - a * m.astype(np.float64)
    return np.where(c < 0.0, c / a, BIG).astype(np.float32)  # -T = c/a


def _prep_shared(inputs):
    d = {k: np.asarray(v, np.float32) for k, v in inputs.items()}

    sw = {i: np.where(d[f"w{i}"] >= 0, 1.0, -1.0).astype(np.float32)
          for i in range(2, 8)}

    wall = np.zeros((128, WTOT), dtype=NP8)

    # L2: [128, 5 pairs, 2, 128]
    blk = wall[:, OFF2:OFF3].reshape(128, 5, 2, 128)
    for k, (ta, tbp) in enumerate(PAIRS):
        blk[:, k, 0, :] = sw[2][ta[0], ta[1]].astype(NP8)
        if tbp is not None:
            blk[:, k, 1, :] = sw[2][tbp[0], tbp[1]].astype(NP8)
    # L3: [128, 2 mc, 5, 2, 128]
    blk = wall[:, OFF3:OFF4].reshape(128, 2, 5, 2, 128)
    for mc in range(2):
        for k, (ta, tbp) in enumerate(PAIRS):
            blk[:, mc, k, 0, :] = sw[3][ta[0], ta[1], :,
                                        mc * 128:(mc + 1) * 128].astype(NP8)
            if tbp is not None:
                blk[:, mc, k, 1, :] = sw[3][tbp[0], tbp[1], :,
                                            mc * 128:(mc + 1) * 128].astype(NP8)
    # L4: [128, 2 mc, 9, 2 kc, 128]
    blk = wall[:, OFF4:OFF5].reshape(128, 2, 9, 2, 128)
    for mc in range(2):
        for t, (dy, dx) in enumerate(TAPS9):
            for kc in range(2):
                blk[:, mc, t, kc, :] = sw[4][
                    dy, dx, kc * 128:(kc + 1) * 128,
                    mc * 128:(mc + 1) * 128].astype(NP8)
    # L5: [128, 4 mc, 9, 2 kc, 128]
    blk = wall[:, OFF5:OFF6].reshape(128, 4, 9, 2, 128)
    for mc in range(4):
        for t, (dy, dx) in enumerate(TAPS9):
            for kc in range(2):
                blk[:, mc, t, kc, :] = sw[5][
                    dy, dx, kc * 128:(kc + 1) * 128,
                    mc * 128:(mc + 1) * 128].astype(NP8)
    # L6: [128, 4 mc, 2 kp, 9, 2 j, 128]
    blk = wall[:, OFF6:OFF7].reshape(128, 4, 2, 9, 2, 128)
    for mc in range(4):
        for kp in range(2):
            for t, (dy, dx) in enumerate(TAPS9):
                for j in range(2):
                    blk[:, mc, kp, t, j, :] = sw[6][
                        dy, dx, (2 * kp + j) * 128:(2 * kp + j + 1) * 128,
                        mc * 128:(mc + 1) * 128].astype(NP8)
    # L7: [128, 2 kp, 16 tap, 2 j, 16] (cols 10..15 zero)
    blk = wall[:, OFF7:].reshape(128, 2, 16, 2, 16)
    for kp in range(2):
        for ty in range(4):
            for tx in range(4):
                for j in range(2):
                    blk[:, kp, ty * 4 + tx, j, :10] = sw[7][
                        ty, tx,
                        (2 * kp + j) * 128:(2 * kp + j + 1) * 128, :].astype(NP8)

    cvecv = np.zeros((128, 14), dtype=np.float32)
    tb1 = _thresh_bias(d["g1"], d["be1"], d["m1"], d["v1"])
    cvecv[:, 0] = (d["b1"].astype(np.float64) + tb1.astype(np.float64)).astype(
        np.float32)
    MCN = {2: 1, 3: 2, 4: 2, 5: 4, 6: 4}
    for layer in (2, 3, 4, 5, 6):
        t = _thresh_bias(
            d[f"g{layer}"], d[f"be{layer}"], d[f"m{layer}"], d[f"v{layer}"])
        cvecv[:, CVCOL[layer]:CVCOL[layer] + MCN[layer]] = t.reshape(
            MCN[layer], 128).T

    a7 = d["g7"].astype(np.float64) / np.sqrt(d["v7"].astype(np.float64) + EPS)
    c7 = d["be7"].astype(np.float64) - a7 * d["m7"].astype(np.float64)
    bn7v = np.stack([a7.astype(np.float32), c7.astype(np.float32)], axis=1)

    # 3-level bf16 split of w1: exact w = w1+w2+w3
    w1m = d["w1"].reshape(27, 128)
    w1a = w1m.astype(NPBF)
    w1b = (w1m - w1a.astype(np.float32)).astype(NPBF)
    w1c = (w1m - w1a.astype(np.float32) - w1b.astype(np.float32)).astype(NPBF)
    w1r = np.zeros((128, 2, 128), dtype=NPBF)
    # MM1 terms x1w1+x1w2+x2w1 ; MM2 terms x2w2+x1w3+x3w1
    w1r[0:27, 0, :] = w1a
    w1r[27:54, 0, :] = w1b
    w1r[54:81, 0, :] = w1a
    w1r[0:27, 1, :] = w1b
    w1r[27:54, 1, :] = w1c
    w1r[54:81, 1, :] = w1a
    return d, wall, cvecv, bn7v, w1r


def _im2col(x):
    """x [NB,32,32,C?] -> [27, grp*y*x*i] rows (dy,dx,c)."""
    from numpy.lib.stride_tricks import sliding_window_view

    nbl = x.shape[0]
    sw = sliding_window_view(x, (3, 3), axis=(1, 2))  # [NB,30,30,3,3,3](c,wy,wx)
    im = sw.transpose(4, 5, 3, 0, 1, 2)  # [wy,wx,c,NB,y,x]
    im = im.reshape(27, nbl // G, G, 30, 30)
    im = im.transpose(0, 1, 3, 4, 2)     # [27, grp, y, x, i]
    return np.ascontiguousarray(im).reshape(27, nbl * 900)


def _xsplit(x):
    """x [NB,32,32,3] f32 -> xa, xb [81, NB*900] bf16 (3-level split ims)."""
    x1 = x.astype(NPBF)
    x2 = (x - x1.astype(np.float32)).astype(NPBF)
    x3 = (x - x1.astype(np.float32) - x2.astype(np.float32)).astype(NPBF)
    i1 = _im2col(x1.astype(np.float32)).astype(NPBF)
    i2 = _im2col(x2.astype(np.float32)).astype(NPBF)
    i3 = _im2col(x3.astype(np.float32)).astype(NPBF)
    xav = np.concatenate([i1, i1, i2], axis=0)   # MM1 moving [x1;x1;x2]
    xbv = np.concatenate([i2, i1, i3], axis=0)   # MM2 moving [x2;x1;x3]
    return xav, xbv


LAST_RESULTS = None


def kernel(**inputs):
    global LAST_RESULTS
    key = (NB, G)
    if key not in _prog_cache:
        _prog_cache[key] = build_program(NB, G)
    nc = _prog_cache[key]

    d, wall, cvecv, bn7v, w1r = _prep_shared(inputs)

    in_maps = []
    for c in range(NCORES):
        xav, xbv = _xsplit(d["x"][c * NB:(c + 1) * NB])
        in_maps.append(
            {"xa": xav, "xb": xbv,
             "xa0": np.ascontiguousarray(xav[:, 0:960]),
             "xb0": np.ascontiguousarray(xbv[:, 0:960]),
             "w1": w1r, "wall": wall, "cvec": cvecv, "bn7": bn7v})

    trace = bool(int(os.environ.get("KERNEL_TRACE", "0")))
    res = run_bass_kernel_spmd(
        nc, in_maps, core_ids=list(range(NCORES)), trace=trace)
    LAST_RESULTS = res
    out = np.concatenate([res.results[i]["y"] for i in range(NCORES)], axis=0)
    return out.astype(np.float32)



# revision 69
# speedup vs baseline: 1.0340x; 1.0086x over previous
"""Trainium2 Bass kernel for the 7-layer Riptide-style binarized CNN.

Data-parallel over 8 NeuronCores, 64 images/core, groups of g=16 images.

The binarized conv layers (L2-L6) run as fp8 DoubleRow matmuls at the
PE's measured stream peak (~154 TF/s, 0.5 cyc per moving element), so
the kernel is compute-bound at the hardware roofline: 2384 DR matmuls
x ~218 ns = ~520 us of a ~609 us kernel, with the Tensor engine >97%%
busy.  L7 runs non-DR (at its tiny free dim DoubleRow's interleaved
LDWEIGHTS load dominates; plain fp8 is ~6x faster per matmul).

Key structural points:
  - All activation grids use a flat [y, x*16imgs] layout (image index
    innermost).  A conv window over (x, i) is then a single contiguous
    512/256/128-col span, so one matmul covers all 16 images of a row
    chunk and post-ops (pool/sign) batch across all images -> far fewer
    Scalar/Vector instructions.
  - L5/L6 compute exact 8x8 output windows (v1 computed the full 10x10
    padded grid, wasting 56%).
  - DoubleRow everywhere: tap pairs are built with *overlapping* moving
    APs (j-dim stride = inter-tap offset into the same buffer), so no
    shifted second copy is stored and the odd tap out of 9 runs as a
    DR pair with zero weights.  All binary-conv matmuls run at the fp8
    DR rate (0.5 cycles/output element).
  - Maxpool via scalar_tensor_tensor(max) between strided views (cost =
    output size, half of reduce_max), on the Vector engine.
  - L7 + BN + softmax run once over all 64 images at the end.
  - Only pad rings are memset (not whole buffers).
"""

import os
import sys

sys.path.insert(0, "/opt/trn_rl_repo")

import numpy as np
import ml_dtypes
from contextlib import ExitStack

import concourse.bass as bass  # noqa: F401
import concourse.mybir as mybir
import concourse.tile as tile
from concourse import bacc
from concourse.ap import AP
from concourse.bass_utils import run_bass_kernel_spmd
from concourse.masks import make_identity

F32 = mybir.dt.float32
F32R = mybir.dt.float32r
BF16 = mybir.dt.bfloat16
NPBF = ml_dtypes.bfloat16
FP8 = mybir.dt.float8e4
NP8 = ml_dtypes.float8_e4m3fn
DR = mybir.MatmulPerfMode.DoubleRow
MAX = mybir.AluOpType.max
MULT = mybir.AluOpType.mult
AX = mybir.AxisListType.X

NCORES = 8
B = 512
NB = B // NCORES
G = 16
EPS = 1e-3
BIG = 1e30

TAPS9 = [(dy, dx) for dy in range(3) for dx in range(3)]

# DR tap pairs for the 1-kc layers (L2, L3): 4 real pairs + 1 zero-padded.
# ((tap_a), (tap_b or None)) ; moving j-stride = offset(tap_b) - offset(tap_a)
PAIRS = [
    ((0, 0), (0, 1)),
    ((1, 1), (1, 2)),
    ((2, 0), (2, 1)),
    ((0, 2), (1, 0)),
    ((2, 2), None),
]

# row widths (elements) of the flat [y, x*16] grids
W2 = 34 * G   # 544
W3 = 18 * G   # 288
W5 = 10 * G   # 160

# weight block offsets inside the packed fp8 buffer [128, WTOT]
OFF2 = 0
OFF3 = OFF2 + 5 * 2 * 128          # 1280
OFF4 = OFF3 + 2 * 5 * 2 * 128      # 3840
OFF5 = OFF4 + 2 * 9 * 2 * 128      # 8448
OFF6 = OFF5 + 4 * 9 * 2 * 128      # 17664
OFF7 = OFF6 + 4 * 2 * 9 * 2 * 128  # 36096
WTOT = OFF7 + 2 * 16 * 2 * 16      # 38144 (L7 q padded 10->16)

CVCOL = {1: 0, 2: 1, 3: 2, 4: 4, 5: 6, 6: 10}

_prog_cache = {}


def _mov(base_view, off, dims):
    """Raw moving AP: dims = [[stride, size], ...] (free dims, no partition)."""
    pitch = base_view.ap[0][0]
    return AP(base_view.tensor, base_view.offset + off, [[pitch, 128]] + dims)


def build_program(nb=NB, g=G, dump=False):
    assert g == 16 and nb % g == 0
    ngrp = nb // g

    nc = bacc.Bacc("TRN2", target_bir_lowering=False, debug=False)
    Sign = mybir.ActivationFunctionType.Sign
    Exp = mybir.ActivationFunctionType.Exp
    Identity = mybir.ActivationFunctionType.Identity

    xa = nc.declare_dram_parameter("xa", [81, nb * 900], BF16, isOutput=False)
    xb = nc.declare_dram_parameter("xb", [81, nb * 900], BF16, isOutput=False)
    # contiguous copies of the first two output rows: a narrow column
    # slice of the flat layout reads 81 tiny strided rows (~16GB/s), so
    # the first l1 chunk would otherwise wait ~10us for strip (0,8)
    xa0 = nc.declare_dram_parameter("xa0", [81, 960], BF16, isOutput=False)
    xb0 = nc.declare_dram_parameter("xb0", [81, 960], BF16, isOutput=False)
    w1 = nc.declare_dram_parameter("w1", [128, 2, 128], BF16, isOutput=False)
    wall = nc.declare_dram_parameter("wall", [128, WTOT], FP8, isOutput=False)
    cvec = nc.declare_dram_parameter("cvec", [128, 14], F32, isOutput=False)
    bn7 = nc.declare_dram_parameter("bn7", [10, 2], F32, isOutput=False)
    y = nc.declare_dram_parameter("y", [nb, 10], F32, isOutput=True)
    if dump:
        d2 = nc.declare_dram_parameter("d2", [128, 34 * W2 + 16], FP8, isOutput=True)
        d3 = nc.declare_dram_parameter("d3", [128, 18 * W3 + 16], FP8, isOutput=True)
        d4 = nc.declare_dram_parameter("d4", [128, 2 * 18 * W3], FP8, isOutput=True)
        d5 = nc.declare_dram_parameter("d5", [128, 2 * 10 * W5], FP8, isOutput=True)
        d6 = nc.declare_dram_parameter("d6", [128, 4 * 10 * W5], FP8, isOutput=True)
        d7 = nc.declare_dram_parameter("d7", [128, 4 * (nb // g) * 256], FP8, isOutput=True)

    with tile.TileContext(nc) as tc, ExitStack() as ctx:
        consts = ctx.enter_context(tc.tile_pool(name="consts", bufs=1))
        sbufs = ctx.enter_context(tc.tile_pool(name="sbufs", bufs=1))
        xpool = ctx.enter_context(tc.tile_pool(name="xpool", bufs=1))
        post = ctx.enter_context(tc.tile_pool(name="post", bufs=4))
        psum = ctx.enter_context(tc.tile_pool(name="psum", bufs=4, space="PSUM"))

        w1sb = consts.tile([128, 2, 128], BF16)
        nc.sync.dma_start(out=w1sb, in_=w1[:, :, :])
        cv = consts.tile([128, 14], F32)
        nc.sync.dma_start(out=cv, in_=cvec[:, :])
        bn7sb = consts.tile([10, 2], F32)
        nc.sync.dma_start(out=bn7sb, in_=bn7[:, :])
        wsb = consts.tile([128, WTOT], FP8)
        ident = consts.tile([10, 10], F32)
        make_identity(nc, ident)

        # ---- weight views
        w2v = wsb[:, OFF2:OFF3].rearrange("p (k j q) -> p k j q", k=5, j=2, q=128)
        w3v = wsb[:, OFF3:OFF4].rearrange(
            "p (m k j q) -> p m k j q", m=2, k=5, j=2, q=128)
        w4v = wsb[:, OFF4:OFF5].rearrange(
            "p (m t j q) -> p m t j q", m=2, t=9, j=2, q=128)
        w5v = wsb[:, OFF5:OFF6].rearrange(
            "p (m t j q) -> p m t j q", m=4, t=9, j=2, q=128)
        w6v = wsb[:, OFF6:OFF7].rearrange(
            "p (m k t j q) -> p m k t j q", m=4, k=2, t=9, j=2, q=128)
        w7v = wsb[:, OFF7:].rearrange(
            "p (k t j q) -> p k t j q", k=2, t=16, j=2, q=16)

        # ---- activation grids, flat [y, x*16] fp8 (+16 slack for the
        # zero-weight dummy DR windows reading 16 past the end)
        s2 = sbufs.tile([128, 34 * W2 + 16], FP8)
        s3 = sbufs.tile([128, 18 * W3 + 16], FP8)
        s4 = sbufs.tile([128, 2, 18 * W3], FP8)
        s5 = sbufs.tile([128, 2, 10 * W5], FP8)
        s6 = sbufs.tile([128, 4, 10 * W5], FP8)
        s7a = sbufs.tile([128, 4, ngrp, 256], FP8)

        # row views for strided interior writes
        s2r = s2[:, : 34 * W2].rearrange("p (y c) -> p y c", y=34)
        s3r = s3[:, : 18 * W3].rearrange("p (y c) -> p y c", y=18)
        s4r = s4.rearrange("p k (y c) -> p k y c", y=18)
        s5r = s5.rearrange("p k (y c) -> p k y c", y=10)
        s6r = s6.rearrange("p k (y c) -> p k y c", y=10)

        # ---- memset pad rings (+1) and slack once.  s2's pads go on the
        # (prologue-idle) Vector engine so the interleaved early L2 chunks
        # don't wait for the gpsimd queue's identity build.
        nc.vector.memset(s2[:, : 2 * W2], 1.0)
        nc.vector.memset(s2[:, 32 * W2:], 1.0)
        nc.gpsimd.memset(s3[:, : W3], 1.0)
        nc.gpsimd.memset(s3[:, 17 * W3:], 1.0)
        nc.gpsimd.memset(s4[:, :, :W3], 1.0)
        nc.gpsimd.memset(s4[:, :, 17 * W3:], 1.0)
        nc.gpsimd.memset(s5[:, :, :W5], 1.0)
        nc.gpsimd.memset(s5[:, :, 9 * W5:], 1.0)
        nc.gpsimd.memset(s6[:, :, :W5], 1.0)
        nc.gpsimd.memset(s6[:, :, 9 * W5:], 1.0)
        # side columns: rows [pad, rows-pad), x < pad or x >= X-pad
        nc.vector.memset(
            s2r[:, 2:32, 0:2 * G], 1.0)
        nc.vector.memset(
            s2r[:, 2:32, 32 * G:], 1.0)
        nc.gpsimd.memset(
            s3r[:, 1:17, 0:G], 1.0)
        nc.gpsimd.memset(
            s3r[:, 1:17, 17 * G:], 1.0)
        for kk in range(2):
            nc.gpsimd.memset(
                s4r[:, kk, 1:17, 0:G], 1.0)
            nc.gpsimd.memset(
                s4r[:, kk, 1:17, 17 * G:], 1.0)
            nc.gpsimd.memset(
                s5r[:, kk, 1:9, 0:G], 1.0)
            nc.gpsimd.memset(
                s5r[:, kk, 1:9, 9 * G:], 1.0)
        for kk in range(4):
            nc.gpsimd.memset(
                s6r[:, kk, 1:9, 0:G], 1.0)
            nc.gpsimd.memset(
                s6r[:, kk, 1:9, 9 * G:], 1.0)

        def tb(layer, mc=0):
            c = CVCOL[layer] + mc
            return cv[:, c:c + 1]

        # strip row ranges for L1 (PE row tiling, K=27 per strip)
        STRIPS = [(0, 8), (8, 16), (16, 24), (24, 30)]

        # tap-pair (offset, jstride) tables for L2 / L3
        def pair_tab(roww):
            tab = []
            for ta, tEb in PAIRS:
                o = ta[0] * roww + ta[1] * G
                if tEb is None:
                    d = G  # dummy: stride anywhere valid; weights are zero
                else:
                    d = tEb[0] * roww + tEb[1] * G - o
                tab.append((o, d))
            return tab

        P2 = pair_tab(W2)
        P3 = pair_tab(W3)

        xbuf = {}

        def xdma(grp):
            xta = xpool.tile([128, 14400], BF16, tag="xa")
            xtb = xpool.tile([128, 14400], BF16, tag="xb")
            xbuf[grp] = (xta, xtb)
            strips = STRIPS
            if grp == 0:
                # rows 0-1 from the dense side tensors (in-flight on
                # separate DMA rings, land ~2us) so chunk 0 starts early
                nc.sync.dma_start(out=xta[0:81, 0:960], in_=xa0[:, :])
                nc.sync.dma_start(out=xtb[0:81, 0:960], in_=xb0[:, :])
                # small w2 block now, before the strips claim the rings:
                # the prologue-interleaved L2 chunks need it by ~7us
                nc.sync.dma_start(out=wsb[:, :OFF3], in_=wall[:, :OFF3])
                strips = [(2, 8)] + STRIPS[1:]
            for (y0, y1) in strips:
                nc.sync.dma_start(
                    out=xta[0:81, y0 * 480:y1 * 480],
                    in_=xa[:, grp * 14400 + y0 * 480: grp * 14400 + y1 * 480],
                )
                nc.sync.dma_start(
                    out=xtb[0:81, y0 * 480:y1 * 480],
                    in_=xb[:, grp * 14400 + y0 * 480: grp * 14400 + y1 * 480],
                )

        def l1_chunk(grp, c):
            # conv1 as exact 3-level bf16 split (2 K=81 matmuls per output
            # row: terms x1w1+x1w2+x2w1, then x2w2+x1w3+x3w1)
            xta, xtb = xbuf[grp]
            p = psum.tile([128, 2, 512], F32, tag="p")
            for r in range(2):
                yy = 2 * c + r
                nc.tensor.matmul(
                    p[:, r, 0:480], w1sb[0:81, 0, :],
                    xta[0:81, yy * 480:(yy + 1) * 480],
                    start=True, stop=False,
                )
                nc.tensor.matmul(
                    p[:, r, 0:480], w1sb[0:81, 1, :],
                    xtb[0:81, yy * 480:(yy + 1) * 480],
                    start=False, stop=True,
                )
            nc.scalar.activation(
                s2r[:, 2 + 2 * c:4 + 2 * c, 2 * G:32 * G],
                p[:, :, 0:480], Sign, bias=tb(1), scale=1.0,
            )

        # prologue: x block DMAs first so L1 starts early; the small L2-L4
        # weight piece next (needed ~20us in); the big remainder after.
        xdma(0)
        nc.sync.dma_start(out=wsb[:, OFF3:OFF5], in_=wall[:, OFF3:OFF5])
        nc.sync.dma_start(out=wsb[:, OFF5:], in_=wall[:, OFF5:])

        # L2: binconv 128->128 (5 DR pairs), pool, sign
        def l2_chunk(grp, c):
            p = psum.tile([128, 2, 512], F32, tag="p", name="p2")
            for r in range(2):
                yy = 2 * c + r
                for k, (o, d) in enumerate(P2):
                    mov = _mov(s2, yy * W2 + o, [[d, 2], [1, 512]])
                    nc.tensor.matmul(
                        p[:, r, :], w2v[:, k, :, :], mov,
                        start=(k == 0), stop=(k == 4), perf_mode=DR,
                    )
            # maxpool 2x2 on raw psum, then sign -> s3 row 1+c interior
            pe = p.rearrange("p y (x two i) -> p y x i two", two=2, i=G)
            t1 = post.tile([128, 2, 256], F32, tag="t1", name="t1c")
            nc.vector.reduce_max(
                t1.rearrange("p y (x i) -> p y x i", i=G), pe, axis=AX)
            t2 = post.tile([128, 256], F32, tag="t2", name="t2c")
            nc.vector.scalar_tensor_tensor(
                t2, t1[:, 0, :], 1.0, t1[:, 1, :], op0=MULT, op1=MAX,
            )
            nc.scalar.activation(
                s3r[:, 1 + c, G:17 * G], t2, Sign, bias=tb(2), scale=1.0,
            )

        # group-0 prologue: interleave L1 with the L2 chunks whose input
        # rows are already signed.  The x transfer (4.7MB, ~24us through
        # the DMA rings) outpaces L1 alone (~16us) but not L1+L2 (~27us),
        # so the PE rides through instead of stalling on strips.
        for c in range(15):
            l1_chunk(0, c)
            if c >= 5:
                l2_chunk(0, c - 5)
        for grp in range(ngrp):
            # chunks 0-9 of group 0's L2 were pre-emitted into the prologue
            for c in range(10 if grp == 0 else 0, 16):
                l2_chunk(grp, c)

            # ---------------- L3: binconv 128->256 (5 DR pairs), sign
            # c-major so L4's first windows (low rows, both kc planes) are
            # signed several chunks before L4 starts.
            for c in range(4):
                for mc in range(2):
                    p = psum.tile([128, 4, 256], F32, tag="p")
                    for h in range(2):
                        yy = 4 * c + 2 * h
                        for k, (o, d) in enumerate(P3):
                            mov = _mov(s3, yy * W3 + o,
                                       [[d, 2], [W3, 2], [1, 256]])
                            nc.tensor.matmul(
                                p[:, 2 * h:2 * h + 2, :], w3v[:, mc, k, :, :],
                                mov, start=(k == 0), stop=(k == 4),
                                perf_mode=DR,
                            )
                    nc.scalar.activation(
                        s4r[:, mc, 1 + 4 * c:5 + 4 * c, G:17 * G],
                        p, Sign, bias=tb(3, mc), scale=1.0,
                    )

            # ---------------- L4: binconv 256->256 (DR over kc), pool, sign
            for c in range(4):
                for mc in range(2):
                    p = psum.tile([128, 4, 256], F32, tag="p")
                    for h in range(2):
                        yy = 4 * c + 2 * h
                        for t, (dy, dx) in enumerate(TAPS9):
                            mov = _mov(s4, (yy + dy) * W3 + dx * G,
                                       [[18 * W3, 2], [W3, 2], [1, 256]])
                            nc.tensor.matmul(
                                p[:, 2 * h:2 * h + 2, :], w4v[:, mc, t, :, :],
                                mov, start=(t == 0), stop=(t == 8),
                                perf_mode=DR,
                            )
                    pe = p.rearrange("p y (x two i) -> p y x i two", two=2, i=G)
                    t1 = post.tile([128, 4, 128], F32, tag="t1")
                    nc.vector.reduce_max(
                        t1.rearrange("p y (x i) -> p y x i", i=G), pe, axis=AX)
                    t1p = t1.rearrange("p (a two) c -> p a two c", two=2)
                    t2 = post.tile([128, 2, 128], F32, tag="t2")
                    nc.vector.scalar_tensor_tensor(
                        t2, t1p[:, :, 0, :], 1.0, t1p[:, :, 1, :],
                        op0=MULT, op1=MAX,
                    )
                    nc.scalar.activation(
                        s5r[:, mc, 1 + 2 * c:3 + 2 * c, G:9 * G],
                        t2, Sign, bias=tb(4, mc), scale=1.0,
                    )

            # ---------------- L5: binconv 256->512 (DR over kc), sign
            for c in range(2):
                for mc in range(4):
                    p = psum.tile([128, 4, 128], F32, tag="p")
                    for t, (dy, dx) in enumerate(TAPS9):
                        mov = _mov(s5, (4 * c + dy) * W5 + dx * G,
                                   [[10 * W5, 2], [W5, 4], [1, 128]])
                        nc.tensor.matmul(
                            p, w5v[:, mc, t, :, :], mov,
                            start=(t == 0), stop=(t == 8), perf_mode=DR,
                        )
                    nc.scalar.activation(
                        s6r[:, mc, 1 + 4 * c:5 + 4 * c, G:9 * G],
                        p, Sign, bias=tb(5, mc), scale=1.0,
                    )

            # ---------------- L6: binconv 512->512 (DR over kc), pool, sign
            # interleave next group's L1 chunks between L6 chunks so the PE
            # keeps streaming through the group boundary while DVE drains.
            if grp + 1 < ngrp:
                xdma(grp + 1)
                pend = [(grp + 1, cc) for cc in range(15)]
            else:
                pend = []
            k6 = 0
            for mc in range(4):
                for c in range(2):
                    p = psum.tile([128, 4, 128], F32, tag="p")
                    k = 0
                    for kp in range(2):
                        for t, (dy, dx) in enumerate(TAPS9):
                            mov = _mov(
                                s6,
                                kp * 2 * 10 * W5 + (4 * c + dy) * W5 + dx * G,
                                [[10 * W5, 2], [W5, 4], [1, 128]])
                            nc.tensor.matmul(
                                p, w6v[:, mc, kp, t, :, :], mov,
                                start=(k == 0), stop=(k == 17), perf_mode=DR,
                            )
                            k += 1
                    pe = p.rearrange("p y (x two i) -> p y x i two", two=2, i=G)
                    t1 = post.tile([128, 4, 64], F32, tag="t1l6")
                    nc.vector.reduce_max(
                        t1.rearrange("p y (x i) -> p y x i", i=G), pe, axis=AX)
                    t1p = t1.rearrange("p (a two) c -> p a two c", two=2)
                    t2 = post.tile([128, 2, 64], F32, tag="t2l6")
                    nc.vector.scalar_tensor_tensor(
                        t2, t1p[:, :, 0, :], 1.0, t1p[:, :, 1, :],
                        op0=MULT, op1=MAX,
                    )
                    nc.scalar.activation(
                        s7a[:, mc, grp, 2 * c * 64:2 * c * 64 + 128],
                        t2, Sign, bias=tb(6, mc), scale=1.0,
                    )
                    k6 += 1
                    if k6 >= 3:
                        for _ in range(3):
                            if pend:
                                l1_chunk(*pend.pop(0))
            while pend:
                l1_chunk(*pend.pop(0))

        if dump:
            for src_t, dst in ((s2, d2), (s3, d3), (s4, d4), (s5, d5),
                               (s6, d6), (s7a, d7)):
                n = src_t.free_size()
                fl = src_t.rearrange(
                    " ".join(["p"] + [chr(97 + i) for i in range(src_t.ndim - 1)])
                    + " -> p (" + " ".join(chr(97 + i) for i in range(src_t.ndim - 1)) + ")"
                ) if src_t.ndim > 2 else src_t
                nc.sync.dma_start(out=dst[:, :], in_=fl)

        # ---------------- L7: binconv 512->10 (4x4) over all 64 images
        p7f = psum.tile([16, nb], F32, tag="p")
        p7 = p7f[0:10, :]
        # non-DR: at this tiny free dim DoubleRow's interleaved LDWEIGHTS
        # dominates (~219 ns/MM); plain fp8 MMs with 16-column weight
        # loads run at ~36 ns/MM.
        k = 0
        for kp in range(2):
            for t in range(16):
                for j in range(2):
                    mov = _mov(s7a, (2 * kp + j) * ngrp * 256 + t * 16,
                               [[256, ngrp], [1, 16]])
                    nc.tensor.matmul(
                        p7f, w7v[:, kp, t, j, :], mov,
                        start=(k == 0), stop=(k == 63),
                    )
                    k += 1
        h7 = post.tile([10, nb], F32, tag="h7")
        nc.vector.tensor_scalar_max(h7, p7, 0.0)
        v7 = post.tile([10, nb], F32, tag="v7")
        nc.scalar.activation(
            v7, h7, Identity, bias=bn7sb[:, 1:2], scale=bn7sb[:, 0:1])
        pt = psum.tile([nb, 10], F32, tag="p")
        nc.tensor.transpose(pt, v7, ident)
        mx = post.tile([nb, 1], F32, tag="mx")
        nc.vector.reduce_max(mx, pt, axis=AX)
        nmx = post.tile([nb, 1], F32, tag="nmx")
        nc.vector.tensor_scalar_mul(nmx, mx, -1.0)
        ex = post.tile([nb, 10], F32, tag="ex")
        nc.scalar.activation(ex, pt, Exp, bias=nmx, scale=1.0)
        sm = post.tile([nb, 1], F32, tag="sm")
        nc.vector.reduce_sum(sm, ex, axis=AX)
        ri = post.tile([nb, 1], F32, tag="ri")
        nc.vector.reciprocal(ri, sm)
        yo = post.tile([nb, 10], F32, tag="yo")
        nc.vector.tensor_scalar_mul(yo, ex, ri)
        nc.sync.dma_start(out=y[:, :], in_=yo)

    nc.compile()
    return nc


# ------------------------------------------------------------------ host prep

def _thresh_bias(gm, be, m, v):
    """bias such that next-layer input = Sign(pre_bn_value + bias)."""
    a = gm.astype(np.float64) / np.sqrt(v.astype(np.float64) + EPS)
    c = be.astype(np.float64) - a * m.astype(np.float64)
    return np.where(c < 0.0, c / a, BIG).astype(np.float32)  # -T = c/a


def _prep_shared(inputs):
    d = {k: np.asarray(v, np.float32) for k, v in inputs.items()}

    sw = {i: np.where(d[f"w{i}"] >= 0, 1.0, -1.0).astype(np.float32)
          for i in range(2, 8)}

    wall = np.zeros((128, WTOT), dtype=NP8)

    # L2: [128, 5 pairs, 2, 128]
    blk = wall[:, OFF2:OFF3].reshape(128, 5, 2, 128)
    for k, (ta, tbp) in enumerate(PAIRS):
        blk[:, k, 0, :] = sw[2][ta[0], ta[1]].astype(NP8)
        if tbp is not None:
            blk[:, k, 1, :] = sw[2][tbp[0], tbp[1]].astype(NP8)
    # L3: [128, 2 mc, 5, 2, 128]
    blk = wall[:, OFF3:OFF4].reshape(128, 2, 5, 2, 128)
    for mc in range(2):
        for k, (ta, tbp) in enumerate(PAIRS):
            blk[:, mc, k, 0, :] = sw[3][ta[0], ta[1], :,
                                        mc * 128:(mc + 1) * 128].astype(NP8)
            if tbp is not None:
                blk[:, mc, k, 1, :] = sw[3][tbp[0], tbp[1], :,
                                            mc * 128:(mc + 1) * 128].astype(NP8)
    # L4: [128, 2 mc, 9, 2 kc, 128]
    blk = wall[:, OFF4:OFF5].reshape(128, 2, 9, 2, 128)
    for mc in range(2):
        for t, (dy, dx) in enumerate(TAPS9):
            for kc in range(2):
                blk[:, mc, t, kc, :] = sw[4][
                    dy, dx, kc * 128:(kc + 1) * 128,
                    mc * 128:(mc + 1) * 128].astype(NP8)
    # L5: [128, 4 mc, 9, 2 kc, 128]
    blk = wall[:, OFF5:OFF6].reshape(128, 4, 9, 2, 128)
    for mc in range(4):
        for t, (dy, dx) in enumerate(TAPS9):
            for kc in range(2):
                blk[:, mc, t, kc, :] = sw[5][
                    dy, dx, kc * 128:(kc + 1) * 128,
                    mc * 128:(mc + 1) * 128].astype(NP8)
    # L6: [128, 4 mc, 2 kp, 9, 2 j, 128]
    blk = wall[:, OFF6:OFF7].reshape(128, 4, 2, 9, 2, 128)
    for mc in range(4):
        for kp in range(2):
            for t, (dy, dx) in enumerate(TAPS9):
                for j in range(2):
                    blk[:, mc, kp, t, j, :] = sw[6][
                        dy, dx, (2 * kp + j) * 128:(2 * kp + j + 1) * 128,
                        mc * 128:(mc + 1) * 128].astype(NP8)
    # L7: [128, 2 kp, 16 tap, 2 j, 16] (cols 10..15 zero)
    blk = wall[:, OFF7:].reshape(128, 2, 16, 2, 16)
    for kp in range(2):
        for ty in range(4):
            for tx in range(4):
                for j in range(2):
                    blk[:, kp, ty * 4 + tx, j, :10] = sw[7][
                        ty, tx,
                        (2 * kp + j) * 128:(2 * kp + j + 1) * 128, :].astype(NP8)

    cvecv = np.zeros((128, 14), dtype=np.float32)
    tb1 = _thresh_bias(d["g1"], d["be1"], d["m1"], d["v1"])
    cvecv[:, 0] = (d["b1"].astype(np.float64) + tb1.astype(np.float64)).astype(
        np.float32)
    MCN = {2: 1, 3: 2, 4: 2, 5: 4, 6: 4}
    for layer in (2, 3, 4, 5, 6):
        t = _thresh_bias(
            d[f"g{layer}"], d[f"be{layer}"], d[f"m{layer}"], d[f"v{layer}"])
        cvecv[:, CVCOL[layer]:CVCOL[layer] + MCN[layer]] = t.reshape(
            MCN[layer], 128).T

    a7 = d["g7"].astype(np.float64) / np.sqrt(d["v7"].astype(np.float64) + EPS)
    c7 = d["be7"].astype(np.float64) - a7 * d["m7"].astype(np.float64)
    bn7v = np.stack([a7.astype(np.float32), c7.astype(np.float32)], axis=1)

    # 3-level bf16 split of w1: exact w = w1+w2+w3
    w1m = d["w1"].reshape(27, 128)
    w1a = w1m.astype(NPBF)
    w1b = (w1m - w1a.astype(np.float32)).astype(NPBF)
    w1c = (w1m - w1a.astype(np.float32) - w1b.astype(np.float32)).astype(NPBF)
    w1r = np.zeros((128, 2, 128), dtype=NPBF)
    # MM1 terms x1w1+x1w2+x2w1 ; MM2 terms x2w2+x1w3+x3w1
    w1r[0:27, 0, :] = w1a
    w1r[27:54, 0, :] = w1b
    w1r[54:81, 0, :] = w1a
    w1r[0:27, 1, :] = w1b
    w1r[27:54, 1, :] = w1c
    w1r[54:81, 1, :] = w1a
    return d, wall, cvecv, bn7v, w1r


def _im2col(x):
    """x [NB,32,32,C?] -> [27, grp*y*x*i] rows (dy,dx,c)."""
    from numpy.lib.stride_tricks import sliding_window_view

    nbl = x.shape[0]
    sw = sliding_window_view(x, (3, 3), axis=(1, 2))  # [NB,30,30,3,3,3](c,wy,wx)
    im = sw.transpose(4, 5, 3, 0, 1, 2)  # [wy,wx,c,NB,y,x]
    im = im.reshape(27, nbl // G, G, 30, 30)
    im = im.transpose(0, 1, 3, 4, 2)     # [27, grp, y, x, i]
    return np.ascontiguousarray(im).reshape(27, nbl * 900)


def _xsplit(x):
    """x [NB,32,32,3] f32 -> xa, xb [81, NB*900] bf16 (3-level split ims)."""
    x1 = x.astype(NPBF)
    x2 = (x - x1.astype(np.float32)).astype(NPBF)
    x3 = (x - x1.astype(np.float32) - x2.astype(np.float32)).astype(NPBF)
    i1 = _im2col(x1.astype(np.float32)).astype(NPBF)
    i2 = _im2col(x2.astype(np.float32)).astype(NPBF)
    i3 = _im2col(x3.astype(np.float32)).astype(NPBF)
    xav = np.concatenate([i1, i1, i2], axis=0)   # MM1 moving [x1;x1;x2]
    xbv = np.concatenate([i2, i1, i3], axis=0)   # MM2 moving [x2;x1;x3]
    return xav, xbv


LAST_RESULTS = None


def kernel(**inputs):
    global LAST_RESULTS
    key = (NB, G)
    if key not in _prog_cache:
        _prog_cache[key] = build_program(NB, G)
    nc = _prog_cache[key]

    d, wall, cvecv, bn7v, w1r = _prep_shared(inputs)

    in_maps = []
    for c in range(NCORES):
        xav, xbv = _xsplit(d["x"][c * NB:(c + 1) * NB])
        in_maps.append(
            {"xa": xav, "xb": xbv,
             "xa0": np.ascontiguousarray(xav[:, 0:960]),
             "xb0": np.ascontiguousarray(xbv[:, 0:960]),
             "w1": w1r, "wall": wall, "cvec": cvecv, "bn7": bn7v})

    trace = bool(int(os.environ.get("KERNEL_TRACE", "0")))
    res = run_bass_kernel_spmd(
        nc, in_maps, core_ids=list(range(NCORES)), trace=trace)
    LAST_RESULTS = res
    out = np.concatenate([res.results[i]["y"] for i in range(NCORES)], axis=0)
    return out.astype(np.float32)

